# revision 10
# baseline (speedup 1.0000x reference)
"""Trainium2 Bass kernel for EnhancedTripletLoss (hard-mining triplet loss).

Strategy (8 NeuronCores, SPMD, no collectives):
  * Rows (anchors) are sharded BY CLASS: core c handles all anchors of class c
    (8 classes == 8 cores), padded to a uniform 128-aligned slab height Mc.
  * Columns (candidates) are permuted into 8 uniform 1024-wide class blocks
    (small classes padded with duplicate columns -- duplicates never change a
    min/max); the excess columns of classes larger than 1024 live in one
    shared OVERFLOW region whose per-class minima become extra bmins columns.
  * Per core, fp8(e4m3) DoubleRow matmuls compute
        g[a, j] = <fp8(-2 e_a), fp8(e_j)> + sqj        (sqj = ||e_j||^2)
    into PSUM: one K=256 DR matmul (both K-chunks packed) plus one K=4 DR
    matmul carrying sqj as four fp8 terms of sq/4 (stationary constant 4.0).
    The core's OWN class columns are sign-negated on the v side, so a single
    free-dim min per block yields both hard-positive and hard-negative stats:
        own block:    min(-g) = -(max over positives of (d2 - sqa))
        other blocks: min( g) =  (min over that block's negatives of (d2-sqa))
    ||e_a||^2 rides per-row in fp32 and is folded in after the reduce.
    fp8 1-term quantization gives loss rel-err ~5e-4 (validated vs fp32 ref).
  * Reduce pipeline is split across engines so the DVE is no longer the
    bottleneck: blocks 0-5 are evacuated fp32->fp16 by the Scalar (Act)
    engine, then min-reduced by a batched DVE tensor_tensor-min halving tree
    (fp16 SBUF hits the DVE 2x packed mode); blocks 6-7 and the overflow
    region are tensor_reduce'd directly from PSUM on the DVE.  fp16 rounding
    commutes with min (monotone), so it only perturbs the final value by
    ~2^-11 relative.
  * All per-anchor-tile epilogue math (block masks, sqrt, margin, masking) is
    deferred to ONE batched pass after the loop.
  * Each core writes per-partition partial sums [128, 2] (loss numerator,
    valid count); the host does the final tiny sum + divide.
"""

import numpy as np
import ml_dtypes

P = 128          # SBUF partitions
D = 256          # embedding dim (fixed by the problem)
NCLS = 8         # number of classes == number of cores
NCORES = 8
MARGIN = 0.3
BIGM = 1.0e30    # block-exclusion additive mask (applied to bmins stats only)
W = 1024         # uniform class-block width: 2 fp32 PSUM banks
NEVAC = 6        # blocks evacuated via Act engine + DVE fp16 tree
SQTERMS = 4      # fp8 terms for the ||e_j||^2 channel (of sq/4, scaled by 4.0)

F8 = ml_dtypes.float8_e4m3


def _layout(counts):
    """Overflow classes/widths from class counts (main blocks are uniform W)."""
    ov = [(c, int(n) - W) for c, n in enumerate(counts) if n > W]
    ovw = sum(w for _, w in ov)
    assert ovw <= 512, f"overflow region too wide: {ovw}"
    return tuple(ov)


def _build_program(Mc, ov):
    import concourse.tile as tile
    from concourse import bacc, mybir

    f32 = mybir.dt.float32
    f16 = mybir.dt.float16
    bf16 = mybir.dt.bfloat16
    fp8 = mybir.dt.float8e4
    AX = mybir.AxisListType.X
    OP = mybir.AluOpType
    DR = mybir.MatmulPerfMode.DoubleRow
    COPY = mybir.ActivationFunctionType.Copy

    Mt = Mc // P
    ovw = sum(w for _, w in ov)
    NB = NCLS + len(ov)
    N = NCLS * W + ovw

    nc = bacc.Bacc("TRN2", target_bir_lowering=False, debug=False)

    vds = [nc.dram_tensor(f"vb{b}", [P, 2 * W], fp8, kind="ExternalInput")
           for b in range(NCLS)]
    if ovw:
        vod = nc.dram_tensor("vov", [P, 2 * ovw], fp8, kind="ExternalInput")
    qd = nc.dram_tensor("q", [2, 2 * N], fp8, kind="ExternalInput")
    ud = nc.dram_tensor("u", [P, 2 * Mc], fp8, kind="ExternalInput")
    sd = nc.dram_tensor("s", [2, 2 * Mc], fp8, kind="ExternalInput")
    sqa = nc.dram_tensor("sqa", [P, Mt], f32, kind="ExternalInput")
    vld = nc.dram_tensor("valid", [P, Mt], f32, kind="ExternalInput")
    pbig = nc.dram_tensor("posbig", [P, Mt * NB], f32, kind="ExternalInput")
    nbig = nc.dram_tensor("negbig", [P, Mt * NB], f32, kind="ExternalInput")
    out = nc.dram_tensor("out", [P, 2], f32, kind="ExternalOutput")

    with tile.TileContext(nc) as tc:
        with (
            tc.tile_pool(name="resident", bufs=1) as res,
            tc.tile_pool(name="psum", bufs=3, space="PSUM") as pp,
            tc.tile_pool(name="povf", bufs=2, space="PSUM") as po,
            tc.tile_pool(name="evac", bufs=2) as ev,
            tc.tile_pool(name="tree", bufs=2) as tr,
            tc.tile_pool(name="epi", bufs=1) as epi,
        ):
            # ---- PE warmup ------------------------------------------------
            # dummy matmuls during the DMA fill so the PE's HAM clock-gate
            # reaches 8/8 (2.4 GHz) before the real stream starts.
            wsrc = res.tile([P, 512], bf16, tag="wsrc")
            nc.vector.memset(wsrc[:], 0.0)
            wp = pp.tile([P, W], f32, tag="pblk", name="warm")
            for _ in range(44):
                nc.tensor.matmul(wp[:, 0:512], wsrc[:, 0:P], wsrc[:, :],
                                 start=True, stop=True)

            # ---- resident loads -------------------------------------------
            dma_engs = [nc.sync, nc.gpsimd, nc.scalar]
            _dma_rr = [0]

            def dma(out_ap, in_ap):
                dma_engs[_dma_rr[0] % len(dma_engs)].dma_start(
                    out=out_ap, in_=in_ap)
                _dma_rr[0] += 1

            ut = res.tile([P, 2, Mc], fp8, tag="u")
            dma(ut[:, :, :], ud[:, :])
            st = res.tile([2, 2, Mc], fp8, tag="s")
            dma(st[:, :, :], sd[:, :])
            sqat = res.tile([P, Mt], f32, tag="sqa")
            dma(sqat[:], sqa[:, :])
            vldt = res.tile([P, Mt], f32, tag="valid")
            dma(vldt[:], vld[:, :])
            pbigt = res.tile([P, Mt, NB], f32, tag="posbig")
            dma(pbigt[:, :, :], pbig[:, :])
            nbigt = res.tile([P, Mt, NB], f32, tag="negbig")
            dma(nbigt[:, :, :], nbig[:, :])
            qt = res.tile([2, 2, N], fp8, tag="q")
            dma(qt[:, :, :], qd[:, :])
            # V moving operand: per-block tiles so block 0 lands first
            vts = []
            for b in range(NCLS):
                t = res.tile([P, 2, W], fp8, tag=f"v{b}", name=f"v{b}")
                dma(t[:, :, :], vds[b][:, :])
                vts.append(t)
            if ovw:
                vot = res.tile([P, 2, ovw], fp8, tag="vov")
                dma(vot[:, :, :], vod[:, :])

            bmall = res.tile([P, Mt, NB], f32, tag="bmall")
            out_sb = res.tile([P, 2], f32, tag="out")

            # ---- main loop ------------------------------------------------
            for mt in range(Mt):
                ms = slice(mt * P, (mt + 1) * P)
                evt = ev.tile([P, NEVAC, W], f16, tag="ev")
                for b in range(NCLS):
                    ptile = pp.tile([P, W], f32, tag="pblk", name="pblk")
                    for s in range(2):
                        cl = slice(s * 512, (s + 1) * 512)
                        cg = slice(b * W + s * 512, b * W + (s + 1) * 512)
                        nc.tensor.matmul(
                            ptile[:, cl], ut[:, :, ms], vts[b][:, :, cl],
                            start=True, stop=False, perf_mode=DR,
                        )
                        nc.tensor.matmul(
                            ptile[:, cl], st[:, :, ms], qt[:, :, cg],
                            start=False, stop=True, perf_mode=DR,
                        )
                    if b < NEVAC:
                        nc.scalar.activation(evt[:, b, :], ptile[:, :], COPY)
                    else:
                        nc.vector.tensor_reduce(
                            bmall[:, mt, b: b + 1], ptile[:, :],
                            axis=AX, op=OP.min,
                        )

                if ovw:
                    otile = po.tile([P, ovw], f32, tag="ovf", name="ovf")
                    og = slice(NCLS * W, NCLS * W + ovw)
                    nc.tensor.matmul(
                        otile[:, :], ut[:, :, ms], vot[:, :, :],
                        start=True, stop=False, perf_mode=DR,
                    )
                    nc.tensor.matmul(
                        otile[:, :], st[:, :, ms], qt[:, :, og],
                        start=False, stop=True, perf_mode=DR,
                    )
                    oo = 0
                    for k, (cls, w) in enumerate(ov):
                        nc.vector.tensor_reduce(
                            bmall[:, mt, NCLS + k: NCLS + k + 1],
                            otile[:, oo:oo + w], axis=AX, op=OP.min,
                        )
                        oo += w

                # fp16 min tree over the evacuated blocks (DVE 2x packed)
                t1 = tr.tile([P, NEVAC, 512], f16, tag="t1")
                nc.vector.tensor_tensor(
                    t1[:, :, :], evt[:, :, 0:512], evt[:, :, 512:1024],
                    op=OP.min)
                t2 = tr.tile([P, NEVAC, 256], f16, tag="t2")
                nc.vector.tensor_tensor(
                    t2[:, :, :], t1[:, :, 0:256], t1[:, :, 256:512],
                    op=OP.min)
                t3 = tr.tile([P, NEVAC, 128], f16, tag="t3")
                nc.vector.tensor_tensor(
                    t3[:, :, :], t2[:, :, 0:128], t2[:, :, 128:256],
                    op=OP.min)
                nc.vector.tensor_reduce(
                    bmall[:, mt, 0:NEVAC], t3[:, :, :], axis=AX, op=OP.min)

            # ---- deferred epilogue (one batched pass) ---------------------
            t8a = epi.tile([P, Mt, NB], f32, tag="t8a")
            nc.vector.tensor_tensor(t8a[:, :, :], bmall[:, :, :],
                                    pbigt[:, :, :], op=OP.add)
            mown = epi.tile([P, Mt], f32, tag="mown")
            nc.vector.tensor_reduce(mown[:], t8a[:, :, :], axis=AX, op=OP.min)

            t8b = epi.tile([P, Mt, NB], f32, tag="t8b")
            nc.vector.tensor_tensor(t8b[:, :, :], bmall[:, :, :],
                                    nbigt[:, :, :], op=OP.add)
            mneg = epi.tile([P, Mt], f32, tag="mneg")
            nc.vector.tensor_reduce(mneg[:], t8b[:, :, :], axis=AX, op=OP.min)

            # pos_d2 = relu(sqa - m_own), neg_d2 = relu(sqa + m_neg), sqrt
            pd2 = epi.tile([P, Mt], f32, tag="pd2")
            nc.vector.scalar_tensor_tensor(
                pd2[:], in0=mown[:], scalar=-1.0, in1=sqat[:],
                op0=OP.mult, op1=OP.add)
            pd2r = epi.tile([P, Mt], f32, tag="pd2r")
            nc.vector.tensor_scalar_max(pd2r[:], pd2[:], 0.0)
            pdists = epi.tile([P, Mt], f32, tag="pdists")
            nc.scalar.sqrt(pdists[:], pd2r[:])

            nd2 = epi.tile([P, Mt], f32, tag="nd2")
            nc.vector.scalar_tensor_tensor(
                nd2[:], in0=mneg[:], scalar=1.0, in1=sqat[:],
                op0=OP.mult, op1=OP.add)
            nd2r = epi.tile([P, Mt], f32, tag="nd2r")
            nc.vector.tensor_scalar_max(nd2r[:], nd2[:], 0.0)
            ndists = epi.tile([P, Mt], f32, tag="ndists")
            nc.scalar.sqrt(ndists[:], nd2r[:])

            per = epi.tile([P, Mt], f32, tag="per")
            nc.vector.scalar_tensor_tensor(
                per[:], in0=pdists[:], scalar=MARGIN, in1=ndists[:],
                op0=OP.add, op1=OP.subtract)
            perr = epi.tile([P, Mt], f32, tag="perr")
            nc.vector.tensor_scalar_max(perr[:], per[:], 0.0)
            num = epi.tile([P, Mt], f32, tag="num")
            nc.vector.tensor_tensor(num[:], perr[:], vldt[:], op=OP.mult)

            nc.vector.tensor_reduce(out_sb[:, 0:1], num[:], axis=AX, op=OP.add)
            nc.vector.tensor_reduce(out_sb[:, 1:2], vldt[:], axis=AX, op=OP.add)
            nc.sync.dma_start(out=out[:, :], in_=out_sb[:])

    nc.compile()
    return nc


def _fp8_terms(x, nterms):
    """Decompose fp32 array into fp8 terms summing to ~x."""
    terms = []
    r = x.astype(np.float32)
    for _ in range(nterms):
        h = r.astype(F8)
        terms.append(h)
        r = r - h.astype(np.float32)
    return terms


def _prepare_inputs(emb, lab):
    """Host-side shard/layout prep.  Returns (in_maps, meta)."""
    B = emb.shape[0]
    assert emb.shape[1] == D
    counts = np.bincount(lab, minlength=NCLS).astype(int)
    assert counts.sum() == B

    order = np.argsort(lab, kind="stable")
    cstart = np.concatenate([[0], np.cumsum(counts)]).astype(int)

    ov = _layout(counts)
    ovw = sum(w for _, w in ov)
    NB = NCLS + len(ov)
    Mc = int(((max(1, counts.max()) + P - 1) // P) * P)
    Mt = Mc // P
    N = NCLS * W + ovw

    sq = np.einsum("ij,ij->i", emb, emb, dtype=np.float32)  # ||e||^2, fp32

    # column index: uniform W-wide main blocks (dup-padded), then overflow
    colidx = np.empty(N, dtype=np.int64)
    own_ranges = {c: [] for c in range(NCLS)}
    for c in range(NCLS):
        idx = order[cstart[c]:cstart[c + 1]][:W]
        if len(idx) == 0:
            idx = order[0:1]  # arbitrary real point; class is invalid anyway
        reps = int(np.ceil(W / len(idx)))
        blk = np.tile(idx, reps)[:W]
        colidx[c * W:(c + 1) * W] = blk
        own_ranges[c].append((c * W, W))
    off = NCLS * W
    for cls, w in ov:
        idx = order[cstart[cls] + W:cstart[cls + 1]]
        assert len(idx) == w
        colidx[off:off + w] = idx
        own_ranges[cls].append((off, w))
        off += w

    # fp8 operands (shared across cores before sign application)
    embT = emb.T  # [256, B]
    v8 = np.ascontiguousarray(
        embT[:, colidx].reshape(2, P, N).transpose(1, 0, 2)
    ).astype(F8)                      # [128, 2, N]: dim = kt*128 + p
    sq_terms = _fp8_terms(sq / 4.0, SQTERMS)
    q8 = np.stack([t[colidx] for t in sq_terms]).reshape(2, 2, N)
    q8 = np.ascontiguousarray(q8.transpose(1, 0, 2))  # [p, kt, N]

    u_full = (-2.0 * emb).astype(F8)  # [B, 256]

    bm_cls = list(range(NCLS)) + [cls for cls, _ in ov]

    in_maps = []
    for c in range(NCLS):
        aidx = order[cstart[c]:cstart[c + 1]]
        if len(aidx) == 0:
            aidx = order[0:1]
        npad = Mc - len(aidx)
        pad = np.full(npad, aidx[0], dtype=np.int64)
        aidx_p = np.concatenate([aidx, pad])

        real = np.zeros(Mc, dtype=np.float32)
        real[: min(len(aidx), Mc)] = 1.0
        cls_valid = 1.0 if (2 <= counts[c] <= B - 1) else 0.0
        valid = (real * cls_valid).reshape(Mt, P).T.copy()  # [128, Mt]

        sqa_t = sq[aidx_p].reshape(Mt, P).T.copy()          # [128, Mt]

        s = np.ones(N, dtype=np.float32)
        for o, w in own_ranges[c]:
            s[o:o + w] = -1.0
        s8 = s.astype(F8)  # +-1 exact

        posbig = np.zeros((P, NB), dtype=np.float32)
        negbig = np.zeros((P, NB), dtype=np.float32)
        for j, bc in enumerate(bm_cls):
            if bc == c:
                negbig[:, j] = BIGM
            else:
                posbig[:, j] = BIGM
        posbig9 = np.tile(posbig[:, None, :], (1, Mt, 1)).reshape(P, Mt * NB)
        negbig9 = np.tile(negbig[:, None, :], (1, Mt, 1)).reshape(P, Mt * NB)

        # u stationary [128, 2, Mc] fp8: dim = kt*128 + p, col = anchor
        uT = u_full[aidx_p].T  # [256, Mc] fp8
        u8 = np.ascontiguousarray(
            uT.reshape(2, P, Mc).transpose(1, 0, 2))
        # sq-channel stationary: constant 4.0 (4 terms of sq/4)
        s_ones = np.full((2, 2, Mc), 4.0, dtype=np.float32).astype(F8)

        vv8 = (v8 * s8[None, None, :]).astype(F8)  # [128, 2, N]
        im = {
            "q": np.ascontiguousarray(
                (q8 * s8[None, None, :]).astype(F8)).reshape(2, 2 * N),
            "u": u8.reshape(P, 2 * Mc),
            "s": s_ones.reshape(2, 2 * Mc),
            "sqa": sqa_t,
            "valid": valid,
            "posbig": posbig9,
            "negbig": negbig9,
        }
        for b in range(NCLS):
            im[f"vb{b}"] = np.ascontiguousarray(
                vv8[:, :, b * W:(b + 1) * W]).reshape(P, 2 * W)
        if ovw:
            im["vov"] = np.ascontiguousarray(
                vv8[:, :, NCLS * W:]).reshape(P, 2 * ovw)
        in_maps.append(im)

    meta = dict(Mc=Mc, ov=ov, Mt=Mt, N=N)
    return in_maps, meta


_PROGRAM_CACHE = {}


def _get_program(Mc, ov):
    key = (Mc, ov)
    if key not in _PROGRAM_CACHE:
        _PROGRAM_CACHE[key] = _build_program(Mc, ov)
    return _PROGRAM_CACHE[key]


def _combine(results):
    num = 0.0
    den = 0.0
    for r in results:
        o = np.asarray(r["out"], dtype=np.float64)
        num += o[:, 0].sum()
        den += o[:, 1].sum()
    return np.float32(num / max(den, 1.0))


def _setup_trace_hook():
    """Register the axon NTFF profile hook if the image lacks antenv.axon_hooks."""
    import sys
    import types
    try:
        from antenv.axon_hooks import get_axon_ntff_profile_hook  # noqa: F401
        return
    except ImportError:
        pass
    import antenv
    from trn_agent_boot.trn_boot import _ntff_profile_via_ctypes

    mod = types.ModuleType("antenv.axon_hooks")
    state = {"h": None}
    mod.set_axon_ntff_profile_hook = lambda h: state.__setitem__("h", h)
    mod.get_axon_ntff_profile_hook = lambda: state["h"]
    sys.modules["antenv.axon_hooks"] = mod
    antenv.axon_hooks = mod
    mod.set_axon_ntff_profile_hook(
        _ntff_profile_via_ctypes("/opt/axon/libaxon_pjrt.so")
    )


def kernel(embeddings, labels, _trace=False):
    emb = np.ascontiguousarray(np.asarray(embeddings, dtype=np.float32))
    lab = np.asarray(labels).astype(np.int64).ravel()

    in_maps, meta = _prepare_inputs(emb, lab)
    nc = _get_program(meta["Mc"], meta["ov"])

    from concourse.bass_utils import run_bass_kernel_spmd

    if _trace:
        _setup_trace_hook()
        import concourse.bass_utils as _bu
        _bu.upload_artifacts = lambda tmpdir: tmpdir  # skip remote upload

    res = run_bass_kernel_spmd(
        nc, in_maps, core_ids=list(range(NCORES)), trace=bool(_trace),
    )
    loss = _combine(res.results)
    if _trace:
        return loss, res
    return loss
